# revision 41
# baseline (speedup 1.0000x reference)
"""Trainium2 Bass kernel for a beam tree-ensemble (256 trees, depth 10,
complete binary trees, 256 features, 8 classes, batch 32768).

Data-parallel over batch across 8 NeuronCores. The full 10-level traversal
runs ON DEVICE:

  For each level d the candidate nodes of all trees form a table of
  M_d = 256 * 2^d (feature, threshold) pairs shared by every sample. The
  feature-value gather x[s, F_d[t, j]] uses the GPSIMD APGather ucode
  (its index list is shared across partitions, which matches the
  sample-independent candidate tables exactly), giving xg[s, (t,j)] for
  all candidates. Then bits = (xg >= TH_d) are selected against a
  one-hot of the current node index (u8 compare/mult/max-reduce), and
  node = 2*node + bit. After 10 levels the u16 leaf-local index
  [0, 1024) is DMA'd out; the host expands leaves to class values (pure
  table lookup on data already resident host-side).

Layout: samples on partitions (32 tiles of 128), trees/candidates on
the free dimension, level tables chunked to 4096 candidates,
double-buffered tile pools so GPSIMD gathers overlap the DVE select.

Transfer engineering: x is shipped as per-feature u16 ranks among that
feature's thresholds (rank_transform; comparison-exact incl. ties) and
widened to f32 on device, halving the dominant upload; leaves are
bit-packed 8x10b -> 5xu16 on device before download; execution goes
through a cached jit of the PJRT custom call (run_device) that
recycles donated output buffers so no zero-fill upload is paid per
call, and uploads the replicated tree tables once instead of per-core.
"""

import sys

sys.path.insert(0, "/opt/trn_rl_repo")

import numpy as np

import concourse.bass as bass
import concourse.tile as tile
from concourse import bacc, mybir, bass_utils
from concourse.alu_op_type import AluOpType
import bass_rust

NUM_TREES = 256
MAX_TREE_DEPTH = 10
NUM_NODES = 2 ** (MAX_TREE_DEPTH + 1) - 1  # 2047
N_INTERNAL = 2 ** MAX_TREE_DEPTH - 1       # 1023
N_FEATURES = 256
N_CLASSES = 8
BATCH = 32768
N_CORES = 8
BC = BATCH // N_CORES                      # 4096 samples per core
P = 128
NTILES = BC // P                           # 32 sample tiles per core
CHUNK = 4096                               # candidates per chunk
NK = 2                                     # sample tiles per DVE op group
SPLIT = 2                                  # pipelined batch splits per call

F32 = mybir.dt.float32
U8 = mybir.dt.uint8
U16 = mybir.dt.uint16
I16 = mybir.dt.int16

_PROGRAM_CACHE = {}


def _split_multi_waits(nc):
    """This walrus build accepts at most one sem-wait per instruction; move
    extra waits onto single-wait NoOps placed before the owner."""
    ctr = 0
    for bb in nc.m.functions[0].blocks:
        new = []
        changed = False
        for inst in bb.instructions:
            si = inst.sync_info
            if si is not None and si.on_wait and len(si.on_wait) > 1:
                waits = list(si.on_wait)
                for w in waits[:-1]:
                    ctr += 1
                    n = mybir.InstNoOp(name=f"WSPLIT-{ctr}", ins=[], outs=[])
                    n.engine = inst.engine
                    n.sync_info = bass_rust.SyncInfo(on_wait=[w], on_update=[])
                    new.append(n)
                si.on_wait = [waits[-1]]
                changed = True
            new.append(inst)
        if changed:
            bb.instructions = new


def _chunk_schedule(T):
    """Yield (level, tree0, trees_per_chunk, chunk_elems) covering every
    internal tree level in table order."""
    sched = []
    for d in range(MAX_TREE_DEPTH):
        Md = T << d
        ch = min(CHUNK, Md)
        tpc = ch >> d
        for c in range(Md // ch):
            sched.append((d, c * tpc, tpc, ch))
    return sched


def build_program(T=NUM_TREES, ntiles=NTILES, split_waits=True):
    bc = ntiles * P
    total = T * N_INTERNAL
    wcols = total // 16

    nc = bacc.Bacc("TRN2", debug=False)
    # x arrives as per-feature 12-bit ranks (host rank_transform + pack_x12,
    # 4 ranks per 3 u16 words); rank compares are exactly equivalent to the
    # f32 compares
    x_d = nc.dram_tensor(
        "xin", [bc, N_FEATURES * 3 // 4], U16, kind="ExternalInput"
    ).ap()
    wf_d = nc.dram_tensor("wf", [16, wcols], I16, kind="ExternalInput").ap()
    th_d = nc.dram_tensor("th", [total], F32, kind="ExternalInput").ap()
    # leaves are 10-bit; 8 leaves pack into 5 u16 words -> T*5/8 per sample
    leaf_d = nc.dram_tensor("leaf", [bc, T * 5 // 8], U16, kind="ExternalOutput").ap()

    with tile.TileContext(nc) as tc:
        with (
            tc.tile_pool(name="res", bufs=1) as res,
            tc.tile_pool(name="thp", bufs=2) as thp,
            tc.tile_pool(name="wfp", bufs=2) as wfp,
            tc.tile_pool(name="xgp", bufs=2) as xgp,
            tc.tile_pool(name="ohp", bufs=2) as ohp,
            tc.tile_pool(name="bbp", bufs=2) as bbp,
        ):
            x_all = res.tile([P, ntiles, N_FEATURES], F32)
            node = res.tile([P, ntiles, T], U16)
            bit = res.tile([P, ntiles, T], U8)
            iota16 = res.tile([P, 512], U16)

            # stage packed 12-bit ranks, unpack to u16, widen to f32 for the
            # ap_gather data window. Groups of 4 ranks from 3 words:
            #   r0 = w0 & 4095
            #   r1 = (w0 >> 12 | w1 << 4) & 4095
            #   r2 = (w1 >> 8  | w2 << 8) & 4095
            #   r3 = w2 >> 4
            W = N_FEATURES * 3 // 4
            xp = xgp.tile([P, ntiles, W], U16, name="xp", tag="xg")
            x16 = xgp.tile([P, ntiles, N_FEATURES], U16, name="x16", tag="xg")
            utmp = ohp.tile([P, ntiles, N_FEATURES // 4], U16, name="utmp", tag="oh")
            nc.sync.dma_start(xp[:], x_d.rearrange("(k p) w -> p k w", p=P))

            def wv(m):  # word m of each 3-word group
                return xp[:].rearrange("p k (g m) -> p k g m", m=3)[:, :, :, m]

            def rv(j):  # rank j of each 4-rank group
                return x16[:].rearrange("p k (g e) -> p k g e", e=4)[:, :, :, j]

            SHL = AluOpType.logical_shift_left
            SHR = AluOpType.logical_shift_right
            AND = AluOpType.bitwise_and
            OR = AluOpType.bitwise_or
            nc.vector.tensor_scalar(rv(0), wv(0), 4095, 0, AND, OR)
            nc.vector.tensor_scalar(utmp[:], wv(0), 12, 0, SHR, OR)
            nc.vector.tensor_scalar(rv(1), wv(1), 4, 0, SHL, OR)
            nc.vector.tensor_tensor(rv(1), rv(1), utmp[:], OR)
            nc.vector.tensor_scalar(rv(1), rv(1), 4095, 0, AND, OR)
            nc.vector.tensor_scalar(utmp[:], wv(1), 8, 0, SHR, OR)
            nc.vector.tensor_scalar(rv(2), wv(2), 8, 0, SHL, OR)
            nc.vector.tensor_tensor(rv(2), rv(2), utmp[:], OR)
            nc.vector.tensor_scalar(rv(2), rv(2), 4095, 0, AND, OR)
            nc.vector.tensor_scalar(rv(3), wv(2), 4, 0, SHR, OR)
            nc.vector.tensor_copy(x_all[:], x16[:])
            nc.gpsimd.memset(node[:], 0)
            nc.gpsimd.iota(
                iota16[:], pattern=[[1, 512]], base=0, channel_multiplier=0,
                allow_small_or_imprecise_dtypes=True,
            )

            woff = 0
            toff = 0
            for d, t0, tpc, ch in _chunk_schedule(T):
                j = 1 << d
                wc = ch // 16
                # wrapped gather indices for this chunk, replicated to the
                # 8 GPSIMD 16-partition groups
                wfb = wfp.tile([P, CHUNK // 16], I16, name=f"wf_{d}_{t0}", tag="wf")
                for g in range(8):
                    nc.sync.dma_start(
                        wfb[16 * g : 16 * (g + 1), :wc], wf_d[:, woff : woff + wc]
                    )
                # thresholds broadcast to all partitions
                thb = thp.tile([P, CHUNK], F32, name=f"th_{d}_{t0}", tag="th")
                nc.sync.dma_start(
                    thb[:, :ch], th_d[toff : toff + ch].partition_broadcast(P)
                )
                for k0 in range(0, ntiles, NK):
                    nkk = min(NK, ntiles - k0)
                    xg = xgp.tile([P, NK, CHUNK], F32, name=f"xg_{d}_{t0}_{k0}", tag="xg")
                    oh = ohp.tile([P, NK, CHUNK], U8, name=f"oh_{d}_{t0}_{k0}", tag="oh")
                    bb = bbp.tile([P, NK, CHUNK], U8, name=f"bb_{d}_{t0}_{k0}", tag="bb")
                    for kk in range(nkk):
                        nc.gpsimd.ap_gather(
                            xg[:, kk, :ch], x_all[:, k0 + kk, :], wfb[:, :wc],
                            channels=P, num_elems=N_FEATURES, d=1, num_idxs=ch,
                        )
                    # bb = (rank_x > rank_th)  <=>  (x >= th)
                    nc.vector.tensor_tensor(
                        bb[:, :nkk, :ch],
                        xg[:, :nkk, :ch],
                        thb[:, :ch].unsqueeze(1).broadcast_to([P, nkk, ch]),
                        AluOpType.is_gt,
                    )
                    # oh = onehot(node == j)
                    nc.vector.tensor_tensor(
                        oh[:, :nkk, :ch].rearrange("p k (t j) -> p k t j", j=j),
                        node[:, k0 : k0 + nkk, t0 : t0 + tpc]
                        .unsqueeze(3)
                        .broadcast_to([P, nkk, tpc, j]),
                        iota16[:, :j]
                        .unsqueeze(1)
                        .unsqueeze(1)
                        .broadcast_to([P, nkk, tpc, j]),
                        AluOpType.is_equal,
                    )
                    # oh *= bb ; bit = max_j oh
                    nc.vector.tensor_tensor(
                        oh[:, :nkk, :ch],
                        oh[:, :nkk, :ch],
                        bb[:, :nkk, :ch],
                        AluOpType.mult,
                    )
                    nc.vector.tensor_reduce(
                        bit[:, k0 : k0 + nkk, t0 : t0 + tpc],
                        oh[:, :nkk, :ch].rearrange("p k (t j) -> p k t j", j=j),
                        axis=mybir.AxisListType.X,
                        op=AluOpType.max,
                    )
                woff += wc
                toff += ch
                if t0 + tpc == T:  # last chunk of this level
                    nc.vector.tensor_scalar(
                        node[:], node[:], 2, 0, AluOpType.mult, AluOpType.add
                    )
                    nc.vector.tensor_tensor(node[:], node[:], bit[:], AluOpType.add)

            # pack 8 consecutive 10-bit leaves into 5 u16 words:
            #   w0 = l0      | l1 << 10
            #   w1 = l1 >> 6 | l2 << 4  | l3 << 14
            #   w2 = l3 >> 2 | l4 << 8
            #   w3 = l4 >> 8 | l5 << 2  | l6 << 12
            #   w4 = l6 >> 4 | l7 << 6
            G = T // 8  # leaf groups per tile-row
            # borrow rotating work-pool slots (gathers are done by now)
            pk = xgp.tile([P, ntiles, G, 5], U16, name="pk", tag="xg")
            tmp = ohp.tile([P, ntiles, G], U16, name="pktmp", tag="oh")

            def lv(i):  # strided view of leaf i within each group of 8
                return node[:].rearrange("p k (g e) -> p k g e", e=8)[:, :, :, i]

            def emit(m, terms):
                # terms: list of (leaf_idx, shift); shift>0 left, <0 right
                dst = pk[:, :, :, m]
                first = True
                for li, sh in terms:
                    if sh == 0:
                        src = lv(li)
                        if first:
                            nc.vector.tensor_copy(dst, src)
                            first = False
                            continue
                        nc.vector.tensor_tensor(dst, dst, src, AluOpType.bitwise_or)
                        continue
                    op = (
                        AluOpType.logical_shift_left
                        if sh > 0
                        else AluOpType.logical_shift_right
                    )
                    nc.vector.tensor_scalar(
                        tmp[:], lv(li), abs(sh), 0, op, AluOpType.bitwise_or
                    )
                    if first:
                        nc.vector.tensor_copy(dst, tmp[:])
                        first = False
                    else:
                        nc.vector.tensor_tensor(
                            dst, dst, tmp[:], AluOpType.bitwise_or
                        )

            emit(0, [(0, 0), (1, 10)])
            emit(1, [(1, -6), (2, 4), (3, 14)])
            emit(2, [(3, -2), (4, 8)])
            emit(3, [(4, -8), (5, 2), (6, 12)])
            emit(4, [(6, -4), (7, 6)])

            nc.sync.dma_start(
                leaf_d.rearrange("(k p) w -> p k w", p=P),
                pk[:].rearrange("p k g m -> p k (g m)"),
            )

    nc.compile()
    if split_waits:  # needed for HW; the sim race detector rejects the NoOps
        _split_multi_waits(nc)
    return nc


_RUNNER_CACHE = {}

# program inputs that are identical on every core (tables); uploaded once
# and replicated device-side instead of 8x over the tunnel
REPLICATED_INPUTS = frozenset({"wf", "th"})


def run_device(nc, full_inputs):
    """Execute the 8-core SPMD program via PJRT like
    bass2jax.run_bass_via_pjrt, with two changes: the jitted executable is
    cached across calls, and the donated output buffers are created on
    device with jnp.zeros instead of being uploaded from the host (the
    kernel writes every output element, so the zero-fill is never
    observable).

    ``full_inputs`` maps tensor name -> global array whose axis 0
    concatenates the per-core shards.
    """
    import jax
    import jax.numpy as jnp
    from jax.sharding import Mesh, PartitionSpec, NamedSharding
    from jax.experimental.shard_map import shard_map
    from concourse import bass2jax as B

    key = id(nc)
    if key not in _RUNNER_CACHE:
        B.install_neuronx_cc_hook()
        partition_name = (
            nc.partition_id_tensor.name if nc.partition_id_tensor else None
        )
        in_names = []
        out_names = []
        out_avals = []
        out_shapes = []
        for alloc in nc.m.functions[0].allocations:
            if not isinstance(alloc, mybir.MemoryLocationSet):
                continue
            name = alloc.memorylocations[0].name
            if alloc.kind == "ExternalInput":
                if name != partition_name:
                    in_names.append(name)
            elif alloc.kind == "ExternalOutput":
                shape = tuple(alloc.tensor_shape)
                dtype = mybir.dt.np(alloc.dtype)
                out_avals.append(jax.core.ShapedArray(shape, dtype))
                out_names.append(name)
                out_shapes.append((shape, dtype))
        n_params = len(in_names)
        n_outs = len(out_names)
        all_names = list(in_names) + list(out_names)
        if partition_name is not None:
            all_names.append(partition_name)

        def _body(*args):
            operands = list(args)
            if partition_name is not None:
                operands.append(B.partition_id_tensor())
            outs = B._bass_exec_p.bind(
                *operands,
                out_avals=tuple(out_avals),
                in_names=tuple(all_names),
                out_names=tuple(out_names),
                lowering_input_output_aliases=(),
                sim_require_finite=True,
                sim_require_nnan=True,
                nc=nc,
            )
            return tuple(outs)

        devices = jax.devices()[:N_CORES]
        mesh = Mesh(np.asarray(devices), ("core",))
        # inputs whose name is in REPLICATED_INPUTS are passed whole to
        # every core (one tunnel upload); the rest shard over axis 0
        specs = tuple(
            PartitionSpec() if n in REPLICATED_INPUTS else PartitionSpec("core")
            for n in in_names
        ) + (PartitionSpec("core"),) * n_outs
        out_specs = (PartitionSpec("core"),) * n_outs
        donate = tuple(range(n_params, n_params + n_outs))
        sharded = jax.jit(
            shard_map(
                _body, mesh=mesh, in_specs=specs, out_specs=out_specs,
                check_rep=False,
            ),
            donate_argnums=donate,
            keep_unused=True,
        )
        sharding = NamedSharding(mesh, PartitionSpec("core"))
        state = {
            "in_names": in_names,
            "out_names": out_names,
            "sharded": sharded,
            "out_shapes": out_shapes,
            "sharding": sharding,
            # output buffers recycled as the next call's donated scratch;
            # every output element is written by the kernel, so contents
            # are irrelevant. Zeros are uploaded only on the first call.
            "bufs": None,
        }
        _RUNNER_CACHE[key] = state

    state = _RUNNER_CACHE[key]
    import jax as _jax

    splits = full_inputs if isinstance(full_inputs, list) else [full_inputs]
    nsplit = len(splits)
    if state["bufs"] is None or len(state["bufs"]) != nsplit:
        state["bufs"] = [
            [
                _jax.device_put(
                    np.zeros((N_CORES * shape[0],) + shape[1:], dtype),
                    state["sharding"],
                )
                for shape, dtype in state["out_shapes"]
            ]
            for _ in range(nsplit)
        ]
    in_names, out_names = state["in_names"], state["out_names"]
    # dispatch every split before materializing any result: split s+1's
    # input upload overlaps split s's device execution
    all_outs = [
        state["sharded"](*[fi[n] for n in in_names], *state["bufs"][s])
        for s, fi in enumerate(splits)
    ]
    results = [
        {name: np.asarray(outs[i]) for i, name in enumerate(out_names)}
        for outs in all_outs
    ]
    state["bufs"] = [list(outs) for outs in all_outs]
    if not isinstance(full_inputs, list):
        return results[0]
    return results


def rank_transform(x, features, thresholds, T=NUM_TREES):
    """Lossless comparison-preserving re-encoding: per feature f, replace
    x[:, f] by its rank among that feature's thresholds (searchsorted
    'right') and each threshold by the first position of its value in the
    sorted list ('left'). Then (x >= th) == (rank_x > rank_th) exactly,
    including ties/duplicates. Shrinks the x upload to u16."""
    feats = features.reshape(T, NUM_NODES)
    thr = thresholds.reshape(T, NUM_NODES)
    fparts, vparts = [], []
    for d, t0, tpc, ch in _chunk_schedule(T):
        lo = (1 << d) - 1
        hi = (2 << d) - 1
        fparts.append(feats[t0 : t0 + tpc, lo:hi].reshape(-1))
        vparts.append(thr[t0 : t0 + tpc, lo:hi].reshape(-1))
    fc = np.concatenate(fparts).astype(np.int64)    # candidate features, chunk order
    vc = np.concatenate(vparts).astype(np.float32)  # candidate thresholds, chunk order
    q = np.empty(fc.size, np.float32)
    xq = np.empty(x.shape, np.uint16)
    for f in range(N_FEATURES):
        m = fc == f
        sv = np.sort(vc[m])
        q[m] = np.searchsorted(sv, vc[m], side="left").astype(np.float32)
        xq[:, f] = np.searchsorted(sv, x[:, f], side="right").astype(np.uint16)
    return xq, q


def pack_x12(xq):
    """Pack u16 ranks (< 4096) 4-per-3-u16: [B, F] -> [B, F*3//4]."""
    B, F = xq.shape
    r = xq.astype(np.uint32).reshape(B, F // 4, 4)
    r0, r1, r2, r3 = (r[:, :, j] for j in range(4))
    w0 = r0 | (r1 << 12)
    w1 = (r1 >> 4) | (r2 << 8)
    w2 = (r2 >> 8) | (r3 << 4)
    return (
        np.stack([w0, w1, w2], axis=2).reshape(B, F * 3 // 4) & 0xFFFF
    ).astype(np.uint16)


def unpack_leaves(packed, T=NUM_TREES):
    """Invert the device 8-leaves->5-u16 packing: [B, T*5//8] u16 -> [B, T]."""
    B = packed.shape[0]
    w = packed.astype(np.uint32).reshape(B, T // 8, 5)
    w0, w1, w2, w3, w4 = (w[:, :, m] for m in range(5))
    ls = [
        w0 & 1023,
        ((w0 >> 10) | (w1 << 6)) & 1023,
        (w1 >> 4) & 1023,
        ((w1 >> 14) | (w2 << 2)) & 1023,
        ((w2 >> 8) | (w3 << 8)) & 1023,
        (w3 >> 2) & 1023,
        ((w3 >> 12) | (w4 << 4)) & 1023,
        (w4 >> 6) & 1023,
    ]
    return np.stack(ls, axis=2).reshape(B, T).astype(np.int64)


def host_tables(features, thresholds, T=NUM_TREES):
    """Per-level (tree-major) threshold table and wrapped feature-index
    blocks matching IndirectCopy's 16-partition interleave."""
    feats = features.reshape(T, NUM_NODES)
    thr = thresholds.reshape(T, NUM_NODES)
    wf_parts = []
    th_parts = []
    for d, t0, tpc, ch in _chunk_schedule(T):
        lo = (1 << d) - 1
        hi = (2 << d) - 1
        Fd = feats[t0 : t0 + tpc, lo:hi].reshape(-1).astype(np.int16)
        Td = thr[t0 : t0 + tpc, lo:hi].reshape(-1).astype(np.float32)
        wf_parts.append(Fd.reshape(ch // 16, 16).T)
        th_parts.append(Td)
    wf = np.ascontiguousarray(np.concatenate(wf_parts, axis=1))
    th = np.concatenate(th_parts)
    return wf, th


def kernel(x, lefts, rights, features, thresholds, values, nodes_offset):
    x = np.asarray(x, dtype=np.float32)
    features = np.asarray(features, dtype=np.int32)
    thresholds = np.asarray(thresholds, dtype=np.float32)
    values = np.asarray(values, dtype=np.float32)

    wf, _th = host_tables(features, thresholds)
    xq, thq = rank_transform(x, features, thresholds)
    xq = pack_x12(xq)

    if "prog" not in _PROGRAM_CACHE:
        _PROGRAM_CACHE["prog"] = build_program(ntiles=NTILES // SPLIT)
    nc = _PROGRAM_CACHE["prog"]

    rows = BATCH // SPLIT
    splits = [
        {
            "xin": np.ascontiguousarray(xq[s * rows : (s + 1) * rows]),
            "wf": wf,
            "th": thq,
        }
        for s in range(SPLIT)
    ]
    res = None
    last_err = None
    for _attempt in range(3):
        try:
            res = run_device(nc, splits)
            break
        except Exception as e:  # transient NRT device-unrecoverable after crashes
            last_err = e
            _RUNNER_CACHE.clear()
    if res is None:
        raise last_err

    leaf = unpack_leaves(
        np.concatenate([r["leaf"] for r in res], axis=0)
    )  # [B, T] leaf-local in [0, 1024)

    vleaf = np.ascontiguousarray(
        values.reshape(NUM_TREES, NUM_NODES, N_CLASSES)[:, N_INTERNAL:, :]
    )
    tix = np.arange(NUM_TREES)[None, :]
    return vleaf[tix, leaf]  # [B, T, 8] float32
